# revision 1
# baseline (speedup 1.0000x reference)
"""Trainium2 Bass kernel for nn_BertEncoder_403726926494.

Reference computation (per batch element):
  - ragged sentence extraction from hidden_states, masked-softmax attention
    pooling per sentence with W_doc            -> doc_pooled [B, D, H]
  - query extraction (rows 1..32), masked-softmax pooling with W_query
    broadcast over D                           -> q_bcast   [B, D, H]

Device strategy (SPMD, one program on 8 cores, 8 batch elements per core):
  - Per core-slot, DMA only the used row-span of hidden_states into SBUF
    (slots are assigned from a global sort of spans so the per-slot span is
    a static program constant shared by all cores).
  - Per-token scores s[t] = x_t . W_doc: DVE/GpSimd tensor_tensor multiply
    against a W-broadcast tile, then a free-dim reduce on ACT (activation
    Copy + accum_out) or DVE (tensor_reduce) -- engine choice per slot to
    balance load.
  - softmax without max-subtraction (scores are O(1)):
      alphaU[t,j] = exp(s[t] + logSel[t,j])   one ACT op per chunk, where
    logSel is a host-built {0, -1e30} mask marking token t in sentence j
    (columns padded to 32 with -1e30).
      num[j,:H] | den[j] = alphaU^T @ [X | 1]  PE matmul with a ones-column
    appended to X; 4 slots share one PSUM tile via tile_position col-groups.
  - out[j] = num[j] / (den[j] + eps)  (eps keeps empty sentences at 0).
  - Query path packs 4 examples x 32 query rows onto 128 partitions; the
    query-length mask and example-block structure fold into one host-built
    log-mask. q_pooled is broadcast over D on the host.
  - b_doc / b_query shift every score in a softmax segment equally, so they
    cancel and are ignored.
"""

import numpy as np

B, L, H = 64, 512, 768
D, S, Q = 16, 64, 32
NCORES = 8
SLOTS = 8
MPAD = 32  # selector columns padded to one PE col-group
NEG_BIAS = -1.0e30
DEN_EPS = 1.0e-30

# Engine assignment knobs (tuned from traces):
#   score TT multiply per slot: "dve" or "gps"
#   score reduce per slot: "act" (per-chunk accum) or "dve" (merged reduce)
TT_ENGINE = ["dve"] * SLOTS
RED_ENGINE = ["act", "act", "act", "act", "act", "act", "dve", "dve"]
Q_RED_ENGINE = "act"

_compiled: dict = {}


def _slot_geometry(slot_spans):
    nts = [(sp + 127) // 128 for sp in slot_spans]
    rems = [sp - 128 * (nt - 1) for sp, nt in zip(slot_spans, nts)]
    coffs = [0]
    for nt in nts:
        coffs.append(coffs[-1] + nt)
    return nts, rems, coffs


def _build(slot_spans):
    """Build + compile the SPMD Bass program for the given per-slot spans."""
    from contextlib import ExitStack

    import concourse.bacc as bacc
    import concourse.tile as tile
    from concourse import mybir

    f32 = mybir.dt.float32
    MULT = mybir.AluOpType.mult
    ADD = mybir.AluOpType.add
    EXP = mybir.ActivationFunctionType.Exp
    COPY = mybir.ActivationFunctionType.Copy

    nts, rems, coffs = _slot_geometry(slot_spans)
    ntsum = coffs[-1]
    foffs = [0]
    for nt in nts:
        foffs.append(foffs[-1] + nt - 1)
    roffs = [0]
    for r in rems:
        roffs.append(roffs[-1] + r)

    nc = bacc.Bacc(
        "TRN2", target_bir_lowering=False, debug=False, num_devices=NCORES
    )
    nfull = sum(nt - 1 for nt in nts)
    nremtot = sum(rems)
    sfull = nc.dram_tensor(
        "sfull", [128, max(nfull, 1), H], f32, kind="ExternalInput"
    ).ap()
    srem = nc.dram_tensor("srem", [nremtot, H], f32, kind="ExternalInput").ap()
    qstage = nc.dram_tensor("qstage", [2, 128, H], f32, kind="ExternalInput").ap()
    wd = nc.dram_tensor("wd", [1, H], f32, kind="ExternalInput").ap()
    wq = nc.dram_tensor("wq", [1, H], f32, kind="ExternalInput").ap()
    selt = nc.dram_tensor(
        "selt", [128, ntsum, MPAD], f32, kind="ExternalInput"
    ).ap()
    qmask = nc.dram_tensor("qmask", [128, 2, MPAD], f32, kind="ExternalInput").ap()
    doc_out = nc.dram_tensor("doc_out", [SLOTS, D, H], f32, kind="ExternalOutput").ap()
    q_out = nc.dram_tensor("q_out", [SLOTS, H], f32, kind="ExternalOutput").ap()

    with tile.TileContext(nc) as tc, ExitStack() as ctx:
        const = ctx.enter_context(tc.tile_pool(name="const", bufs=1))

        wrow_d = const.tile([1, H], f32)
        nc.sync.dma_start(out=wrow_d[:], in_=wd[:])
        wrow_q = const.tile([1, H], f32)
        nc.sync.dma_start(out=wrow_q[:], in_=wq[:])
        selt_t = const.tile([128, ntsum, MPAD], f32)
        nc.sync.dma_start(out=selt_t[:], in_=selt[:])
        qmask_t = const.tile([128, 2, MPAD], f32)
        nc.sync.dma_start(out=qmask_t[:], in_=qmask[:])

        # Broadcast W rows across all 128 partitions (gpsimd custom op).
        wb_d = const.tile([128, H], f32)
        wb_q = const.tile([128, H], f32)
        nc.gpsimd.partition_broadcast(wb_d[:], wrow_d[:])
        nc.gpsimd.partition_broadcast(wb_q[:], wrow_q[:])

        xpool = ctx.enter_context(tc.tile_pool(name="xp", bufs=8))
        apool = ctx.enter_context(tc.tile_pool(name="apl", bufs=4))
        scrp = ctx.enter_context(tc.tile_pool(name="scr", bufs=2))
        outp = ctx.enter_context(tc.tile_pool(name="outp", bufs=2))
        smallp = ctx.enter_context(tc.tile_pool(name="smallp", bufs=4))
        qpoolp = ctx.enter_context(tc.tile_pool(name="qpl", bufs=2))
        nump = ctx.enter_context(tc.tile_pool(name="nump", bufs=2, space="PSUM"))
        qnump = ctx.enter_context(tc.tile_pool(name="qnump", bufs=1, space="PSUM"))

        # ---- scores: xw = x * W_bcast (TT), then free-dim reduce -> scol ----
        def emit_scores(x_ap_full, nt, rem, scol, wb, name, tt_eng, red_eng):
            # x_ap_full: [128, nt, H(+1)] view; uses cols 0:H
            xw = scrp.tile([128, nt, H], f32, tag="scratch", name=f"xw{name}")
            tt = nc.gpsimd if tt_eng == "gps" else nc.vector
            if nt > 1:
                tt.tensor_tensor(
                    out=xw[:, 0 : nt - 1, :],
                    in0=x_ap_full[:, 0 : nt - 1, 0:H],
                    in1=wb[:].rearrange("p (o h) -> p o h", o=1).broadcast_to(
                        [128, nt - 1, H]
                    ),
                    op=MULT,
                )
            tt.tensor_tensor(
                out=xw[0:rem, nt - 1, :],
                in0=x_ap_full[0:rem, nt - 1, 0:H],
                in1=wb[0:rem, :],
                op=MULT,
            )
            if red_eng == "dve":
                if nt > 1:
                    nc.vector.tensor_reduce(
                        out=scol[:, 0 : nt - 1],
                        in_=xw[:, 0 : nt - 1, :],
                        axis=mybir.AxisListType.X,
                        op=ADD,
                    )
                nc.vector.tensor_reduce(
                    out=scol[0:rem, nt - 1 : nt],
                    in_=xw[0:rem, nt - 1, :],
                    axis=mybir.AxisListType.X,
                    op=ADD,
                )
            else:
                s2 = scrp.tile([128, H], f32, tag="scratch2", name=f"s2{name}")
                for c in range(nt):
                    cnt = 128 if c < nt - 1 else rem
                    nc.scalar.activation(
                        s2[0:cnt, :], xw[0:cnt, c, :], COPY,
                        bias=0.0, scale=1.0,
                        accum_out=scol[0:cnt, c : c + 1],
                    )

        # ---- doc slots: per-slot pipeline; two groups of 4 share PSUM tiles
        # via PE col-groups. Slots are emitted alternating between the two
        # groups so independent work overlaps and consecutive slots' matmuls
        # land on different col-groups (concurrent PE streams).
        numgs = {}

        xtiles = {}

        def load_slot(s):
            nt, rem = nts[s], rems[s]
            x = xpool.tile([128, nt, H + 1], f32, tag="x", name=f"x{s}")
            if nt > 1:
                nc.sync.dma_start(
                    out=x[:, 0 : nt - 1, 0:H],
                    in_=sfull[:, foffs[s] : foffs[s] + nt - 1, :],
                )
            nc.sync.dma_start(
                out=x[0:rem, nt - 1, 0:H],
                in_=srem[roffs[s] : roffs[s] + rem, :],
            )
            nc.vector.memset(x[:, :, H : H + 1], 1.0)
            xtiles[s] = x

        def emit_slot(s):
            g, k = divmod(s, 4)
            if g not in numgs:
                numgs[g] = nump.tile([128, 1024], f32, tag="num", name=f"num{g}")
            numg = numgs[g]
            nt, rem, coff = nts[s], rems[s], coffs[s]
            x = xtiles[s]

            scol = smallp.tile([128, nt], f32, tag="scol", name=f"scol{s}")
            emit_scores(
                x[:], nt, rem, scol, wb_d, f"d{s}", TT_ENGINE[s], RED_ENGINE[s]
            )

            at = apool.tile([128, nt, MPAD], f32, tag="at", name=f"at{s}")
            for c in range(nt):
                cnt = 128 if c < nt - 1 else rem
                nc.scalar.activation(
                    at[0:cnt, c, :],
                    selt_t[0:cnt, coff + c, :],
                    EXP,
                    bias=scol[0:cnt, c : c + 1],
                    scale=1.0,
                )
            for c in range(nt):
                cnt = 128 if c < nt - 1 else rem
                first, last = c == 0, c == nt - 1
                nc.tensor.matmul(
                    numg[32 * k : 32 * k + MPAD, 0:512],
                    at[0:cnt, c, :],
                    x[0:cnt, c, 0:512],
                    start=first, stop=last,
                    tile_position=(0, 32 * k),
                    skip_group_check=True,
                )
                nc.tensor.matmul(
                    numg[32 * k : 32 * k + MPAD, 512 : H + 1],
                    at[0:cnt, c, :],
                    x[0:cnt, c, 512 : H + 1],
                    start=first, stop=last,
                    tile_position=(0, 32 * k),
                    skip_group_check=True,
                )

        def finish_group(g):
            numg = numgs[g]
            de = smallp.tile([128, 1], f32, tag="de", name=f"de{g}")
            nc.vector.tensor_scalar(
                out=de[:], in0=numg[:, H : H + 1], scalar1=DEN_EPS,
                scalar2=None, op0=ADD,
            )
            rec = smallp.tile([128, 1], f32, tag="rec", name=f"rec{g}")
            nc.vector.reciprocal(rec[:], de[:])
            do = outp.tile([128, H], f32, tag="do", name=f"do{g}")
            nc.scalar.activation(
                do[:], numg[:, 0:H], COPY, bias=0.0, scale=rec[:, 0:1]
            )
            for k in range(4):
                nc.scalar.dma_start(
                    out=doc_out[4 * g + k, :, :],
                    in_=do[32 * k : 32 * k + D, :],
                )

        # ---- query: two batches of 4 examples x 32 rows -> one PSUM tile ----
        def emit_query(qnumg, b):
            qpack = qpoolp.tile([128, H + 1], f32, tag="qpack", name=f"qpack{b}")
            nc.sync.dma_start(out=qpack[:, 0:H], in_=qstage[b, :, :])
            nc.vector.memset(qpack[:, H : H + 1], 1.0)
            qscol = smallp.tile([128, 1], f32, tag="qscol", name=f"qscol{b}")
            emit_scores(
                qpack[:].rearrange("p (o h) -> p o h", o=1), 1, 128, qscol, wb_q,
                f"q{b}", "dve", Q_RED_ENGINE,
            )
            qat = apool.tile([128, MPAD], f32, tag="qat", name=f"qat{b}")
            nc.scalar.activation(
                qat[:], qmask_t[:, b, :], EXP, bias=qscol[:, 0:1], scale=1.0
            )
            nc.tensor.matmul(
                qnumg[32 * b : 32 * b + MPAD, 0:512],
                qat[:], qpack[:, 0:512],
                start=True, stop=True, tile_position=(0, 32 * b),
            )
            nc.tensor.matmul(
                qnumg[32 * b : 32 * b + MPAD, 512 : H + 1],
                qat[:], qpack[:, 512 : H + 1],
                start=True, stop=True, tile_position=(0, 32 * b),
            )

        qnumg = qnump.tile([64, 1024], f32, tag="qnum", name="qnum")
        for s in range(SLOTS):
            load_slot(s)
        for s in (0, 4, 1, 5):
            emit_slot(s)
        emit_query(qnumg, 0)
        for s in (2, 6, 3, 7):
            emit_slot(s)
        emit_query(qnumg, 1)
        finish_group(0)
        finish_group(1)

        qde = smallp.tile([64, 1], f32, tag="qde", name="qde")
        nc.vector.tensor_scalar(
            out=qde[:], in0=qnumg[:, H : H + 1], scalar1=DEN_EPS,
            scalar2=None, op0=ADD,
        )
        qrec = smallp.tile([64, 1], f32, tag="qrec", name="qrec")
        nc.vector.reciprocal(qrec[:], qde[:])
        qo = outp.tile([64, H], f32, tag="qo", name="qo")
        nc.scalar.activation(
            qo[:], qnumg[:, 0:H], COPY, bias=0.0, scale=qrec[:, 0:1]
        )
        for b in range(2):
            nc.sync.dma_start(
                out=q_out[4 * b : 4 * b + 4, :],
                in_=qo[32 * b : 32 * b + 4, :],
            )

    nc.compile()
    return nc


def _prepare(query_len, seq_lens):
    """Host-side geometry: spans, slot assignment, selector/mask arrays."""
    ql = np.asarray(query_len).astype(np.int64)
    sl = np.asarray(seq_lens).astype(np.int64)
    offs = ql[:, None] + 2 + np.cumsum(sl, axis=1) - sl  # [B, D] sentence starts
    end = ql + 2 + sl.sum(axis=1)
    span = np.maximum(end, 1 + Q)  # query rows 1..32 must be covered
    order = np.argsort(-span, kind="stable")  # rank -> example id
    slot_spans = tuple(int(span[order[8 * s]]) for s in range(SLOTS))
    nts, rems, coffs = _slot_geometry(slot_spans)
    ntsum = coffs[-1]

    selt_all = np.full((NCORES, 128, ntsum, MPAD), NEG_BIAS, np.float32)
    qmask_all = np.full((NCORES, 128, 2, MPAD), NEG_BIAS, np.float32)
    ex_map = np.empty((NCORES, SLOTS), np.int64)
    for c in range(NCORES):
        for s in range(SLOTS):
            e = int(order[8 * s + c])
            ex_map[c, s] = e
            for j in range(D):
                ln = int(sl[e, j])
                if ln == 0:
                    continue
                o = int(offs[e, j])
                t = np.arange(o, o + ln)
                selt_all[c, t % 128, coffs[s] + t // 128, j] = 0.0
            b, sub = divmod(s, 4)
            qmask_all[c, 32 * sub : 32 * sub + int(ql[e]), b, sub] = 0.0
    return slot_spans, ex_map, selt_all, qmask_all


def kernel(hidden_states, W_doc, b_doc, W_query, b_query, query_len, seq_lens):
    hs = np.ascontiguousarray(np.asarray(hidden_states, dtype=np.float32))
    wd = np.ascontiguousarray(np.asarray(W_doc, np.float32).reshape(1, H))
    wq = np.ascontiguousarray(np.asarray(W_query, np.float32).reshape(1, H))

    slot_spans, ex_map, selt_all, qmask_all = _prepare(query_len, seq_lens)

    nc = _compiled.get(slot_spans)
    if nc is None:
        nc = _build(slot_spans)
        _compiled[slot_spans] = nc

    nts, rems, _ = _slot_geometry(slot_spans)
    nfull = sum(nt - 1 for nt in nts)
    nremtot = sum(rems)

    in_maps = []
    for c in range(NCORES):
        sfull = np.empty((128, max(nfull, 1), H), np.float32)
        srem = np.empty((nremtot, H), np.float32)
        qstage = np.empty((2, 128, H), np.float32)
        fo = ro = 0
        for s in range(SLOTS):
            e = int(ex_map[c, s])
            nt, rem = nts[s], rems[s]
            if nt > 1:
                sfull[:, fo : fo + nt - 1, :] = (
                    hs[e, 0 : (nt - 1) * 128, :]
                    .reshape(nt - 1, 128, H)
                    .transpose(1, 0, 2)
                )
                fo += nt - 1
            srem[ro : ro + rem] = hs[e, (nt - 1) * 128 : (nt - 1) * 128 + rem, :]
            ro += rem
            b, sub = divmod(s, 4)
            qstage[b, 32 * sub : 32 * sub + 32, :] = hs[e, 1 : 1 + Q, :]
        in_maps.append(
            {
                "sfull": sfull,
                "srem": srem,
                "qstage": qstage,
                "wd": wd,
                "wq": wq,
                "selt": selt_all[c],
                "qmask": qmask_all[c],
            }
        )

    from concourse.bass_utils import run_bass_kernel_spmd

    res = run_bass_kernel_spmd(nc, in_maps, list(range(NCORES)))

    doc = np.empty((B, D, H), np.float32)
    qp = np.empty((B, H), np.float32)
    for c in range(NCORES):
        r = res.results[c]
        for s in range(SLOTS):
            e = int(ex_map[c, s])
            doc[e] = r["doc_out"][s]
            qp[e] = r["q_out"][s]
    q_bcast = np.broadcast_to(qp[:, None, :], (B, D, H))
    return doc, q_bcast



# revision 18
# speedup vs baseline: 2.0421x; 2.0421x over previous
"""Trainium2 Bass kernel for nn_BertEncoder_403726926494.

Reference computation (per batch element):
  - ragged sentence extraction from hidden_states, masked-softmax attention
    pooling per sentence with W_doc            -> doc_pooled [B, D, H]
  - query extraction (rows 1..32), masked-softmax pooling with W_query
    broadcast over D                           -> q_bcast   [B, D, H]

Device strategy (SPMD, one program on 8 cores, 8 batch elements per core):
  - All float traffic staged host-side as bf16 (tolerance is 2e-2; bf16
    keeps us ~100x under it).  PE matmuls run 4x faster than f32, DMA
    moves half the bytes.
  - The 8 per-core examples (slots) are split into two PSUM groups of 4.
    Each group's token rows are concatenated into one dense stream and
    chunked into [128, 769] tiles (768 hidden + a baked ones column for
    the softmax denominator).  Per chunk:
      score  s[t] = x_t . W_doc      one fused multiply+reduce op
                                     (DVE tensor_tensor_reduce or GpSimd
                                     scalar_tensor_tensor accum), engine
                                     chosen per chunk to balance load
      alphaU[t,j] = exp(s[t]+sel)    one ACT op against a host-built
                                     {0,-1e30} selector [128 cols = 4
                                     slots x 32 sentence cols]
      num|den += alphaU^T @ [X|1]    one PE matmul pair (N=512 + N=257)
                                     accumulating over the group chunks
  - out[j] = num[j] / (den[j] + eps); empty sentences stay 0.
  - Query path: all 8 examples' rows 1..32 pack into 2 chunks of 128;
    an M=8 stationary pools every example in one matmul chain.
  - W rows are staged pre-broadcast to 128 partitions (no gpsimd
    broadcast on the critical path).  b_doc / b_query shift all scores
    in a softmax segment equally and cancel; ignored.
  - Outputs return as bf16 and are upcast on the host.
"""

import numpy as np
import ml_dtypes

B, L, H = 64, 512, 768
D, S, Q = 16, 64, 32
NCORES = 8
SLOTS = 8
HP = H + 1  # ones column appended
NEG_BIAS = -1.0e30
DEN_EPS = 1.0e-30
BF16 = ml_dtypes.bfloat16

# Per-score-op engine: "dve" (tensor_tensor_reduce) or "gps"
# (scalar_tensor_tensor + accum).  Order matches emission order of score
# ops: interleaved group chunks then the two query chunks.
SCORE_ENG_DOC = ["dstt"] * 32  # [g0c0, g1c0, g0c1, g1c1, ...]
SCORE_ENG_Q = ["dstt", "dstt"]
# x stream DMA slicing per group (chunks per slice)
SLICE_PAT = [1, 3, 3]
# Debug: emission stages (0=DMA only, 1=+scores, 2=+exp, 3=full)
BUILD_STAGE = 3

_compiled: dict = {}


def _geometry(slot_spans):
    """Greedy-balanced split of the 8 slots into two groups of 4; chunk
    counts per group stream."""
    order = sorted(range(SLOTS), key=lambda s: -slot_spans[s])
    groups = [[], []]
    sums = [0, 0]
    for s in order:
        g = 0 if (sums[0] <= sums[1] and len(groups[0]) < 4) or len(groups[1]) >= 4 else 1
        groups[g].append(s)
        sums[g] += slot_spans[s]
    ncks = [(sm + 127) // 128 for sm in sums]
    cbase = [0, ncks[0]]
    return groups, sums, ncks, cbase


def _slices(nck):
    out = []
    c = 0
    pat = list(SLICE_PAT)
    while c < nck:
        n = min(pat.pop(0) if pat else 3, nck - c)
        out.append((c, n))
        c += n
    return out


def _build(slot_spans):
    """Build + compile the SPMD Bass program for the given per-slot spans."""
    from contextlib import ExitStack

    import concourse.bacc as bacc
    import concourse.tile as tile
    from concourse import mybir

    f32 = mybir.dt.float32
    bf16 = mybir.dt.bfloat16
    MULT = mybir.AluOpType.mult
    ADD = mybir.AluOpType.add
    EXP = mybir.ActivationFunctionType.Exp
    COPY = mybir.ActivationFunctionType.Copy

    groups, sums, ncks, cbase = _geometry(slot_spans)
    totc = ncks[0] + ncks[1]

    nc = bacc.Bacc(
        "TRN2", target_bir_lowering=False, debug=False, num_devices=NCORES
    )
    xs = nc.dram_tensor("xs", [128, totc, HP], bf16, kind="ExternalInput").ap()
    selt = nc.dram_tensor("selt", [128, totc, 128], bf16, kind="ExternalInput").ap()
    qstage = nc.dram_tensor("qstage", [128, 2, HP], bf16, kind="ExternalInput").ap()
    qmask = nc.dram_tensor("qmask", [128, 2, 8], bf16, kind="ExternalInput").ap()
    wb = nc.dram_tensor("wb", [128, 2, H], bf16, kind="ExternalInput").ap()
    doc_out = nc.dram_tensor("doc_out", [SLOTS, D, H], bf16, kind="ExternalOutput").ap()
    q_out = nc.dram_tensor("q_out", [SLOTS, H], bf16, kind="ExternalOutput").ap()

    with tile.TileContext(nc) as tc, ExitStack() as ctx:
        const = ctx.enter_context(tc.tile_pool(name="const", bufs=1))
        xpool = ctx.enter_context(tc.tile_pool(name="xp", bufs=1))
        apool = ctx.enter_context(tc.tile_pool(name="apl", bufs=6))
        scrp = ctx.enter_context(tc.tile_pool(name="scr", bufs=1))
        outp = ctx.enter_context(tc.tile_pool(name="outp", bufs=1))
        smallp = ctx.enter_context(tc.tile_pool(name="smallp", bufs=1))
        nump = ctx.enter_context(tc.tile_pool(name="nump", bufs=1, space="PSUM"))
        qnump = ctx.enter_context(tc.tile_pool(name="qnump", bufs=1, space="PSUM"))

        # ---- input DMAs, spread across issue queues ----
        qrr = [nc.sync, nc.gpsimd, nc.scalar, nc.sync]

        wb_t = const.tile([128, 2, H], bf16)
        nc.sync.dma_start(out=wb_t[:], in_=wb[:])
        selt_t = const.tile([128, totc, 128], bf16)
        nc.scalar.dma_start(out=selt_t[:], in_=selt[:])
        qmask_t = const.tile([128, 2, 8], bf16)
        nc.gpsimd.dma_start(out=qmask_t[:], in_=qmask[:])

        # x stream slice tiles: xsl[g] -> list of (c0, n, tile)
        xsl = [[], []]
        slices = [_slices(ncks[0]), _slices(ncks[1])]
        qi = 0
        for i in range(max(len(slices[0]), len(slices[1]))):
            for g in (0, 1):
                if i >= len(slices[g]):
                    continue
                c0, n = slices[g][i]
                t = xpool.tile(
                    [128, n, HP], bf16, tag=f"x{g}_{c0}", name=f"x{g}_{c0}"
                )
                qrr[qi % 4].dma_start(
                    out=t[:], in_=xs[:, cbase[g] + c0 : cbase[g] + c0 + n, :]
                )
                qi += 1
                xsl[g].append((c0, n, t))

        qp_t = const.tile([128, 2, HP], bf16)
        qrr[qi % 4].dma_start(out=qp_t[:], in_=qstage[:])

        def xchunk(g, c):
            for c0, n, t in xsl[g]:
                if c0 <= c < c0 + n:
                    return t[:, c - c0, :]
            raise AssertionError

        # ---- per-chunk score -> exp -> matmul ----
        scols = [
            smallp.tile([128, ncks[g]], f32, tag=f"scol{g}", name=f"scol{g}")
            for g in (0, 1)
        ]
        junk_dve = scrp.tile([128, H], bf16, tag="jd", name="jd")
        junk_gps = scrp.tile([128, H], bf16, tag="jg", name="jg")
        junk2 = scrp.tile([128, H], bf16, tag="j2", name="j2")

        def emit_score(x_ap, scol_ap, eng, w=0):
            if eng == "dve":
                nc.vector.tensor_tensor_reduce(
                    out=junk_dve[:],
                    in0=x_ap,
                    in1=wb_t[:, w, :],
                    scale=1.0,
                    scalar=0.0,
                    op0=MULT,
                    op1=ADD,
                    accum_out=scol_ap,
                )
                return
            if eng == "dstt":
                nc.vector.scalar_tensor_tensor(
                    out=junk_dve[:],
                    in0=x_ap,
                    scalar=1.0,
                    in1=wb_t[:, w, :],
                    op0=MULT,
                    op1=MULT,
                    accum_out=scol_ap,
                )
                return
            if eng == "dtr":
                nc.vector.tensor_tensor(
                    out=junk_dve[:], in0=x_ap, in1=wb_t[:, w, :], op=MULT
                )
                nc.vector.tensor_reduce(
                    out=scol_ap, in_=junk_dve[:],
                    axis=mybir.AxisListType.X, op=ADD,
                )
                return
            # gps/dve TT multiply followed by gps/act reduce
            tt = nc.gpsimd if eng in ("gtt", "gact") else nc.vector
            junk = junk_gps if eng in ("gtt", "gact") else junk_dve
            tt.tensor_tensor(out=junk[:], in0=x_ap, in1=wb_t[:, w, :], op=MULT)
            if eng == "gtt":
                nc.gpsimd.tensor_reduce(
                    out=scol_ap, in_=junk[:], axis=mybir.AxisListType.X, op=ADD
                )
            else:
                nc.scalar.activation(
                    junk2[:], junk[:], COPY, bias=0.0, scale=1.0,
                    accum_out=scol_ap,
                )

        numgs = [
            nump.tile([128, 1024], f32, tag=f"num{g}", name=f"num{g}")
            for g in (0, 1)
        ]

        si = 0

        def emit_chunk(g, c):
            nonlocal si
            x = xchunk(g, c)
            if BUILD_STAGE < 1:
                return
            emit_score(x[:, 0:H], scols[g][:, c : c + 1], SCORE_ENG_DOC[si])
            si += 1
            if BUILD_STAGE < 2:
                return
            at = apool.tile([128, 128], bf16, tag="at", name=f"at{g}_{c}")
            nc.scalar.activation(
                at[:], selt_t[:, cbase[g] + c, :], EXP,
                bias=scols[g][:, c : c + 1], scale=1.0,
            )
            if BUILD_STAGE < 3:
                return
            first, last = c == 0, c == ncks[g] - 1
            nc.tensor.matmul(
                numgs[g][:, 0:512], at[:], x[:, 0:512],
                start=first, stop=last, skip_group_check=True,
            )
            nc.tensor.matmul(
                numgs[g][:, 512:HP], at[:], x[:, 512:HP],
                start=first, stop=last, skip_group_check=True,
            )

        qnum = qnump.tile([8, 1024], f32, tag="qnum", name="qnum")
        qscol = smallp.tile([128, 2], f32, tag="qscol", name="qscol")

        def emit_query(b):
            emit_score(qp_t[:, b, 0:H], qscol[:, b : b + 1], SCORE_ENG_Q[b], w=1)
            qat = apool.tile([128, 8], bf16, tag="qat", name=f"qat{b}")
            nc.scalar.activation(
                qat[:], qmask_t[:, b, :], EXP, bias=qscol[:, b : b + 1], scale=1.0
            )
            nc.tensor.matmul(
                qnum[:, 0:512], qat[:], qp_t[:, b, 0:512],
                start=b == 0, stop=b == 1, skip_group_check=True,
            )
            nc.tensor.matmul(
                qnum[:, 512:HP], qat[:], qp_t[:, b, 512:HP],
                start=b == 0, stop=b == 1, skip_group_check=True,
            )

        def finish_group(g):
            numg = numgs[g]
            rec = smallp.tile([128, 1], f32, tag=f"rec{g}", name=f"rec{g}")
            de = smallp.tile([128, 1], f32, tag=f"de{g}", name=f"de{g}")
            nc.vector.tensor_scalar(
                out=de[:], in0=numg[:, H:HP], scalar1=DEN_EPS, scalar2=None,
                op0=ADD,
            )
            nc.vector.reciprocal(rec[:], de[:])
            do = outp.tile([128, H], bf16, tag=f"do{g}", name=f"do{g}")
            nc.scalar.activation(
                do[:], numg[:, 0:H], COPY, bias=0.0, scale=rec[:, 0:1]
            )
            for k, s in enumerate(groups[g]):
                qrr[(k + 2 * g) % 4].dma_start(
                    out=doc_out[s, :, :], in_=do[32 * k : 32 * k + D, :]
                )

        # ---- emission schedule: interleave the two group pipelines ----
        nmax = max(ncks)
        emitted_q = 0
        for c in range(nmax):
            for g in (0, 1):
                if c < ncks[g]:
                    emit_chunk(g, c)
            if c == 1 and BUILD_STAGE >= 3:
                emit_query(0)
                emit_query(1)
                emitted_q = 1
        if BUILD_STAGE >= 3:
            if not emitted_q:
                emit_query(0)
                emit_query(1)

            finish_group(0)
            finish_group(1)

            qde = smallp.tile([8, 1], f32, tag="qde", name="qde")
            nc.vector.tensor_scalar(
                out=qde[:], in0=qnum[:, H:HP], scalar1=DEN_EPS, scalar2=None,
                op0=ADD,
            )
            qrec = smallp.tile([8, 1], f32, tag="qrec", name="qrec")
            nc.vector.reciprocal(qrec[:], qde[:])
            qo = outp.tile([8, H], bf16, tag="qo", name="qo")
            nc.scalar.activation(
                qo[:], qnum[:, 0:H], COPY, bias=0.0, scale=qrec[:, 0:1]
            )
            nc.sync.dma_start(out=q_out[:], in_=qo[:])
        else:
            zo = outp.tile([128, H], bf16, tag="zo", name="zo")
            nc.vector.memset(zo[:], 0.0)
            for s in range(SLOTS):
                nc.sync.dma_start(out=doc_out[s, :, :], in_=zo[0:D, :])
            nc.scalar.dma_start(out=q_out[:], in_=zo[0:SLOTS, :])

    nc.compile()
    return nc


def _prepare(query_len, seq_lens):
    """Host-side geometry: spans, slot assignment, per-core staged arrays.

    Returns (slot_spans, ex_map, stages) where stages is a list of per-core
    dicts of bf16 input arrays for the device program.
    """
    ql = np.asarray(query_len).astype(np.int64)
    sl = np.asarray(seq_lens).astype(np.int64)
    offs = ql[:, None] + 2 + np.cumsum(sl, axis=1) - sl  # [B, D] sentence starts
    end = ql + 2 + sl.sum(axis=1)
    span = np.maximum(end, 1 + Q)  # query rows 1..32 must be covered
    order = np.argsort(-span, kind="stable")  # rank -> example id
    slot_spans = tuple(int(span[order[8 * s]]) for s in range(SLOTS))
    groups, sums, ncks, cbase = _geometry(slot_spans)
    totc = ncks[0] + ncks[1]

    ex_map = np.empty((NCORES, SLOTS), np.int64)
    for c in range(NCORES):
        for s in range(SLOTS):
            ex_map[c, s] = int(order[8 * s + c])

    return slot_spans, ex_map, (groups, sums, ncks, cbase, totc, offs, ql, sl)


def _stage_core(hs, c, slot_spans, ex_map, geo):
    groups, sums, ncks, cbase, totc, offs, ql, sl = geo
    xs32 = np.zeros((128, totc, HP), np.float32)
    selt32 = np.full((128, totc, 128), NEG_BIAS, np.float32)
    qstage32 = np.zeros((128, 2, HP), np.float32)
    qmask32 = np.full((128, 2, 8), NEG_BIAS, np.float32)
    xs32[:, :, H] = 1.0
    qstage32[:, :, H] = 1.0

    for g in (0, 1):
        qoff = 0
        for k, s in enumerate(groups[g]):
            e = int(ex_map[c, s])
            spn = slot_spans[s]
            idx = np.arange(qoff, qoff + spn)
            p, ck = idx % 128, idx // 128
            xs32[p, cbase[g] + ck, 0:H] = hs[e, 0:spn, :]
            for j in range(D):
                ln = int(sl[e, j])
                if ln == 0:
                    continue
                o = int(offs[e, j])
                t = np.arange(qoff + o, qoff + o + ln)
                selt32[t % 128, cbase[g] + t // 128, 32 * k + j] = 0.0
            # query rows 1..32 of this example at qstage[:, g], block k
            qstage32[32 * k : 32 * k + 32, g, 0:H] = hs[e, 1 : 1 + Q, :]
            qmask32[32 * k + np.arange(int(ql[e])), g, 4 * g + k] = 0.0
            qoff += spn
    return {
        "xs": xs32.astype(BF16),
        "selt": selt32.astype(BF16),
        "qstage": qstage32.astype(BF16),
        "qmask": qmask32.astype(BF16),
    }


def kernel(hidden_states, W_doc, b_doc, W_query, b_query, query_len, seq_lens):
    hs = np.ascontiguousarray(np.asarray(hidden_states, dtype=np.float32))
    wd = np.asarray(W_doc, np.float32).reshape(H)
    wq = np.asarray(W_query, np.float32).reshape(H)

    slot_spans, ex_map, geo = _prepare(query_len, seq_lens)
    groups = geo[0]

    nc = _compiled.get(slot_spans)
    if nc is None:
        nc = _build(slot_spans)
        _compiled[slot_spans] = nc

    wb = np.empty((128, 2, H), np.float32)
    wb[:, 0, :] = wd[None, :]
    wb[:, 1, :] = wq[None, :]
    wb = wb.astype(BF16)

    in_maps = []
    for c in range(NCORES):
        m = _stage_core(hs, c, slot_spans, ex_map, geo)
        m["wb"] = wb
        in_maps.append(m)

    from concourse.bass_utils import run_bass_kernel_spmd

    res = run_bass_kernel_spmd(nc, in_maps, list(range(NCORES)))

    doc = np.empty((B, D, H), np.float32)
    qp = np.empty((B, H), np.float32)
    for c in range(NCORES):
        r = res.results[c]
        dout = np.asarray(r["doc_out"], dtype=np.float32)
        qout = np.asarray(r["q_out"], dtype=np.float32)
        for g in (0, 1):
            for k, s in enumerate(groups[g]):
                e = int(ex_map[c, s])
                doc[e] = dout[s]
                qp[e] = qout[4 * g + k]
    q_bcast = np.broadcast_to(qp[:, None, :], (B, D, H))
    return doc, q_bcast


# revision 23
# speedup vs baseline: 2.1942x; 1.0745x over previous
"""Trainium2 Bass kernel for nn_BertEncoder_403726926494.

Reference computation (per batch element):
  - ragged sentence extraction from hidden_states, masked-softmax attention
    pooling per sentence with W_doc            -> doc_pooled [B, D, H]
  - query extraction (rows 1..32), masked-softmax pooling with W_query
    broadcast over D                           -> q_bcast   [B, D, H]

Device strategy (SPMD, one program on 8 cores, 8 batch elements per core):
  - All float traffic staged host-side as bf16 (tolerance is 2e-2; bf16
    keeps us ~5x under it).  PE matmuls run 4x faster than f32, DMA moves
    half the bytes.
  - The 8 per-core examples (slots) are concatenated into ONE dense token
    stream and chunked into [128, 769] tiles (768 hidden + a baked ones
    column for the softmax denominator).  8 slots x 16 sentences = 128
    selector columns = one full-width PE stationary.  Per chunk:
      score  s[t] = x_t . W_doc      one fused DVE scalar_tensor_tensor
                                     with accum_out (single pass)
      alphaU[t,c] = exp(s[t]+sel)    one ACT op against a host-built
                                     {0,-1e30} selector [128 cols]
      num|den += alphaU^T @ [X|1]    one PE matmul pair (N=512 + N=257)
                                     accumulating over all 13 chunks
  - out[16s+j] = num/(den+eps); one [128,768] scale, one contiguous
    doc_out DMA.  Empty sentences stay 0.
  - Query path: all 8 examples' rows 1..32 pack into 2 chunks of 128;
    an M=8 stationary pools every example in one 2-matmul chain.
  - W rows are staged pre-broadcast to 128 partitions.  b_doc / b_query
    shift every score in a softmax segment equally and cancel; ignored.
  - Outputs return as bf16 and are upcast on the host.

Hardware notes (learned the hard way):
  - tensor_tensor_reduce faults the device (NRT unrecoverable); DVE
    scalar_tensor_tensor with accum_out is the working fused op.
  - scalar_tensor_tensor is not supported by codegen on GpSimd.
  - dma_start may only issue from sync/scalar/gpsimd queues.
"""

import numpy as np
import ml_dtypes

B, L, H = 64, 512, 768
D, S, Q = 16, 64, 32
NCORES = 8
SLOTS = 8
HP = H + 1  # ones column appended
NEG_BIAS = -1.0e30
DEN_EPS = 1.0e-30
BF16 = ml_dtypes.bfloat16

# Per-score-op engine (chunks then queries): "dstt" (fused DVE op),
# "dtr" (DVE TT + DVE reduce), "gact" (GpSimd TT + ACT accum reduce)
SCORE_ENG_DOC = ["dstt"] * 32
SCORE_ENG_Q = ["dstt", "dstt"]
# x stream DMA slicing (chunks per slice)
SLICE_PAT = [1, 3, 3, 3, 3, 3, 3]
# Emit query score ops after this chunk index
Q_AFTER = 3
BUILD_STAGE = 3  # debug: 0=DMA only, 1=+scores, 2=+exp, 3=full

_compiled: dict = {}


def _geometry(slot_spans):
    offs = [0]
    for sp in slot_spans:
        offs.append(offs[-1] + sp)
    tot = offs[-1]
    nck = (tot + 127) // 128
    return offs, tot, nck


def _slices(nck):
    out = []
    c = 0
    pat = list(SLICE_PAT)
    while c < nck:
        n = min(pat.pop(0) if pat else 3, nck - c)
        out.append((c, n))
        c += n
    return out


def _build(slot_spans):
    """Build + compile the SPMD Bass program for the given per-slot spans."""
    from contextlib import ExitStack

    import concourse.bacc as bacc
    import concourse.tile as tile
    from concourse import mybir

    f32 = mybir.dt.float32
    bf16 = mybir.dt.bfloat16
    MULT = mybir.AluOpType.mult
    ADD = mybir.AluOpType.add
    EXP = mybir.ActivationFunctionType.Exp
    COPY = mybir.ActivationFunctionType.Copy

    offs, tot, nck = _geometry(slot_spans)
    slices = _slices(nck)

    nc = bacc.Bacc(
        "TRN2", target_bir_lowering=False, debug=False, num_devices=NCORES
    )
    xs = nc.dram_tensor("xs", [128, nck, HP], bf16, kind="ExternalInput").ap()
    selt = nc.dram_tensor("selt", [128, nck, 128], bf16, kind="ExternalInput").ap()
    qstage = nc.dram_tensor("qstage", [128, 2, HP], bf16, kind="ExternalInput").ap()
    qmask = nc.dram_tensor("qmask", [128, 2, 8], bf16, kind="ExternalInput").ap()
    wbd = nc.dram_tensor("wbd", [128, H], bf16, kind="ExternalInput").ap()
    wbq = nc.dram_tensor("wbq", [128, H], bf16, kind="ExternalInput").ap()
    doc_out = nc.dram_tensor(
        "doc_out", [SLOTS * D, H], bf16, kind="ExternalOutput"
    ).ap()
    q_out = nc.dram_tensor("q_out", [SLOTS, H], bf16, kind="ExternalOutput").ap()

    with tile.TileContext(nc) as tc, ExitStack() as ctx:
        const = ctx.enter_context(tc.tile_pool(name="const", bufs=1))
        xpool = ctx.enter_context(tc.tile_pool(name="xp", bufs=1))
        apool = ctx.enter_context(tc.tile_pool(name="apl", bufs=4))
        work = ctx.enter_context(tc.tile_pool(name="work", bufs=1))
        nump = ctx.enter_context(tc.tile_pool(name="nump", bufs=1, space="PSUM"))

        # ---- input DMAs: x stream first (pipeline-critical), spread over
        # the three DMA-capable queues; metadata interleaved by first use.
        wbd_t = const.tile([128, H], bf16)
        nc.sync.dma_start(out=wbd_t[:], in_=wbd[:])
        selt_t = const.tile([128, nck, 128], bf16)
        nc.scalar.dma_start(out=selt_t[:], in_=selt[:])

        xts = []
        qrr = [nc.sync, nc.gpsimd, nc.gpsimd, nc.sync]
        for i, (c0, n) in enumerate(slices):
            t = xpool.tile([128, n, HP], bf16, tag=f"x{c0}", name=f"x{c0}")
            qrr[i % len(qrr)].dma_start(out=t[:], in_=xs[:, c0 : c0 + n, :])
            xts.append((c0, n, t))

        wbq_t = const.tile([128, H], bf16)
        nc.gpsimd.dma_start(out=wbq_t[:], in_=wbq[:])
        qp_t = const.tile([128, 2, HP], bf16)
        nc.scalar.dma_start(out=qp_t[:], in_=qstage[:])
        qmask_t = const.tile([128, 2, 8], bf16)
        nc.sync.dma_start(out=qmask_t[:], in_=qmask[:])

        def xchunk(c):
            for c0, n, t in xts:
                if c0 <= c < c0 + n:
                    return t[:, c - c0, :]
            raise AssertionError

        # ---- per-chunk score -> exp -> matmul ----
        scol = work.tile([128, nck], f32, tag="scol", name="scol")
        qscol = work.tile([128, 2], f32, tag="qscol", name="qscol")
        junk_dve = work.tile([128, H], bf16, tag="jd", name="jd")
        junk_gps = work.tile([128, H], bf16, tag="jg", name="jg")
        junk2 = work.tile([128, H], bf16, tag="j2", name="j2")

        def emit_score(x_ap, scol_ap, eng, wb):
            if eng == "dstt":
                nc.vector.scalar_tensor_tensor(
                    out=junk_dve[:], in0=x_ap, scalar=1.0, in1=wb,
                    op0=MULT, op1=MULT, accum_out=scol_ap,
                )
            elif eng == "dtr":
                nc.vector.tensor_tensor(
                    out=junk_dve[:], in0=x_ap, in1=wb, op=MULT
                )
                nc.vector.tensor_reduce(
                    out=scol_ap, in_=junk_dve[:],
                    axis=mybir.AxisListType.X, op=ADD,
                )
            else:  # gact
                nc.gpsimd.tensor_tensor(
                    out=junk_gps[:], in0=x_ap, in1=wb, op=MULT
                )
                nc.scalar.activation(
                    junk2[:], junk_gps[:], COPY, bias=0.0, scale=1.0,
                    accum_out=scol_ap,
                )

        numg = nump.tile([128, 1024], f32, tag="num", name="num")
        qnum = nump.tile([8, 1024], f32, tag="qnum", name="qnum")

        def emit_chunk(c):
            x = xchunk(c)
            if BUILD_STAGE < 1:
                return
            emit_score(x[:, 0:H], scol[:, c : c + 1], SCORE_ENG_DOC[c], wbd_t[:])
            if BUILD_STAGE < 2:
                return
            at = apool.tile([128, 128], bf16, tag="at", name=f"at{c}")
            nc.scalar.activation(
                at[:], selt_t[:, c, :], EXP, bias=scol[:, c : c + 1], scale=1.0
            )
            if BUILD_STAGE < 3:
                return
            first, last = c == 0, c == nck - 1
            nc.tensor.matmul(
                numg[:, 0:512], at[:], x[:, 0:512],
                start=first, stop=last, skip_group_check=True,
            )
            nc.tensor.matmul(
                numg[:, 512:HP], at[:], x[:, 512:HP],
                start=first, stop=last, skip_group_check=True,
            )

        def emit_query(b):
            emit_score(
                qp_t[:, b, 0:H], qscol[:, b : b + 1], SCORE_ENG_Q[b], wbq_t[:]
            )
            qat = apool.tile([128, 8], bf16, tag="qat", name=f"qat{b}")
            nc.scalar.activation(
                qat[:], qmask_t[:, b, :], EXP, bias=qscol[:, b : b + 1], scale=1.0
            )
            nc.tensor.matmul(
                qnum[:, 0:512], qat[:], qp_t[:, b, 0:512],
                start=b == 0, stop=b == 1, skip_group_check=True,
            )
            nc.tensor.matmul(
                qnum[:, 512:HP], qat[:], qp_t[:, b, 512:HP],
                start=b == 0, stop=b == 1, skip_group_check=True,
            )

        for c in range(nck):
            emit_chunk(c)
            if c == Q_AFTER and BUILD_STAGE >= 3:
                emit_query(0)
                emit_query(1)

        if BUILD_STAGE >= 3:
            # ---- finish: doc scale on ACT, query scale on DVE (parallel) ----
            de = work.tile([128, 1], f32, tag="de", name="de")
            nc.vector.tensor_scalar(
                out=de[:], in0=numg[:, H:HP], scalar1=DEN_EPS, scalar2=None,
                op0=ADD,
            )
            rec = work.tile([128, 1], f32, tag="rec", name="rec")
            nc.vector.reciprocal(rec[:], de[:])
            do = work.tile([128, H], bf16, tag="do", name="do")
            nc.scalar.activation(do[:], numg[:, 0:H], COPY, bias=0.0, scale=rec[:, 0:1])
            nc.sync.dma_start(out=doc_out[:, :], in_=do[:])

            qde = work.tile([8, 1], f32, tag="qde", name="qde")
            nc.vector.tensor_scalar(
                out=qde[:], in0=qnum[:, H:HP], scalar1=DEN_EPS, scalar2=None,
                op0=ADD,
            )
            qrec = work.tile([8, 1], f32, tag="qrec", name="qrec")
            nc.vector.reciprocal(qrec[:], qde[:])
            qo = work.tile([8, H], bf16, tag="qo", name="qo")
            nc.vector.tensor_scalar(
                out=qo[:], in0=qnum[:, 0:H], scalar1=qrec[:, 0:1], scalar2=None,
                op0=MULT,
            )
            nc.scalar.dma_start(out=q_out[:], in_=qo[:])
        else:
            zo = work.tile([128, H], bf16, tag="zo", name="zo")
            nc.vector.memset(zo[:], 0.0)
            nc.sync.dma_start(out=doc_out[:, :], in_=zo[:])
            nc.scalar.dma_start(out=q_out[:], in_=zo[0:SLOTS, :])

    nc.compile()
    return nc


def _prepare(query_len, seq_lens):
    """Host-side geometry: spans, slot assignment (rank-sorted)."""
    ql = np.asarray(query_len).astype(np.int64)
    sl = np.asarray(seq_lens).astype(np.int64)
    offs = ql[:, None] + 2 + np.cumsum(sl, axis=1) - sl  # [B, D] sentence starts
    end = ql + 2 + sl.sum(axis=1)
    span = np.maximum(end, 1 + Q)  # query rows 1..32 must be covered
    order = np.argsort(-span, kind="stable")  # rank -> example id
    slot_spans = tuple(int(span[order[8 * s]]) for s in range(SLOTS))
    ex_map = np.empty((NCORES, SLOTS), np.int64)
    for c in range(NCORES):
        for s in range(SLOTS):
            ex_map[c, s] = int(order[8 * s + c])
    return slot_spans, ex_map, (offs, ql, sl)


def _stage_core(hs, c, slot_spans, ex_map, geo):
    soffs, tot, nck = _geometry(slot_spans)
    offs, ql, sl = geo
    xs32 = np.zeros((128, nck, HP), np.float32)
    selt32 = np.full((128, nck, 128), NEG_BIAS, np.float32)
    qstage32 = np.zeros((128, 2, HP), np.float32)
    qmask32 = np.full((128, 2, 8), NEG_BIAS, np.float32)
    xs32[:, :, H] = 1.0
    qstage32[:, :, H] = 1.0

    for s in range(SLOTS):
        e = int(ex_map[c, s])
        spn = slot_spans[s]
        qoff = soffs[s]
        idx = np.arange(qoff, qoff + spn)
        xs32[idx % 128, idx // 128, 0:H] = hs[e, 0:spn, :]
        for j in range(D):
            ln = int(sl[e, j])
            if ln == 0:
                continue
            o = int(offs[e, j])
            t = np.arange(qoff + o, qoff + o + ln)
            selt32[t % 128, t // 128, 16 * s + j] = 0.0
        b, k = divmod(s, 4)
        qstage32[32 * k : 32 * k + 32, b, 0:H] = hs[e, 1 : 1 + Q, :]
        qmask32[32 * k + np.arange(int(ql[e])), b, s] = 0.0
    return {
        "xs": xs32.astype(BF16),
        "selt": selt32.astype(BF16),
        "qstage": qstage32.astype(BF16),
        "qmask": qmask32.astype(BF16),
    }


def kernel(hidden_states, W_doc, b_doc, W_query, b_query, query_len, seq_lens):
    hs = np.ascontiguousarray(np.asarray(hidden_states, dtype=np.float32))
    wd = np.asarray(W_doc, np.float32).reshape(H)
    wq = np.asarray(W_query, np.float32).reshape(H)

    slot_spans, ex_map, geo = _prepare(query_len, seq_lens)

    nc = _compiled.get(slot_spans)
    if nc is None:
        nc = _build(slot_spans)
        _compiled[slot_spans] = nc

    wbd = np.broadcast_to(wd[None, :], (128, H)).astype(BF16)
    wbq = np.broadcast_to(wq[None, :], (128, H)).astype(BF16)

    in_maps = []
    for c in range(NCORES):
        m = _stage_core(hs, c, slot_spans, ex_map, geo)
        m["wbd"] = wbd
        m["wbq"] = wbq
        in_maps.append(m)

    from concourse.bass_utils import run_bass_kernel_spmd

    res = run_bass_kernel_spmd(nc, in_maps, list(range(NCORES)))

    doc = np.empty((B, D, H), np.float32)
    qp = np.empty((B, H), np.float32)
    for c in range(NCORES):
        r = res.results[c]
        dout = np.asarray(r["doc_out"], dtype=np.float32).reshape(SLOTS, D, H)
        qout = np.asarray(r["q_out"], dtype=np.float32)
        for s in range(SLOTS):
            e = int(ex_map[c, s])
            doc[e] = dout[s]
            qp[e] = qout[s]
    q_bcast = np.broadcast_to(qp[:, None, :], (B, D, H))
    return doc, q_bcast
